# revision 61
# baseline (speedup 1.0000x reference)
"""Trainium2 Bass kernel for a bare KAN layer (PCHIP spline mixing).

Math: out[b, o] = sum_d f_{o,d}(x[b,d]) + bias[o], where f_{o,d} is the PCHIP
cubic interpolant of coeffs[o,d,:] on K=64 uniform knots over [-2, 2], with
linear extrapolation outside.

Device strategy (per core, data-parallel over batch):
  Segment-power telescoping basis. With t = (x - X_MIN)/h and
  u_s = clamp(t - s, 0, 1) for segments s = 0..K-2:

      f(t) = f(0) + sum_s g_s(u_s),   g_s(u) = b_s u + c_s u^2 + d_s u^3

  because each g_s vanishes at u=0 and the u=1 plateaus telescope to
  f(floor) - f(0) exactly; linear extrapolation outside the domain is the
  extra  -hS_0*relu(-t) + hS_{K-1}*relu(t-(K-1))  term.

  Per group of 128 rows (64 dims x 2 segments) the fields are built with
  four engine ops -- y = ACT Identity(t - s) (fp32->fp16), u = DVE
  clamp(y,0,1) (4x mode), then either u2/u3 fp16 multiplies (DVE/Pool) or,
  for half the groups, localized fields q = u(u-1), r = q*u written as
  fp8-e4m3 and contracted with a DoubleRow matmul (0.5 cycles/row) against
  fp8 tables (c+d, d) -- q,r vanish on both plateaus, so fp8 error only
  touches the active segment. All fields accumulate into fp32 PSUM. t is
  replicated [t;t] host-side, so there is no per-group broadcast matmul.

Self-contained: hardcodes shapes B=8192, D=64, K=64, O=64, 8 cores.
"""

import sys

import numpy as np

sys.path.insert(0, "/opt/trn_rl_repo")

from concourse import bass, mybir  # noqa: E402
from concourse.bass_utils import run_bass_kernel_spmd  # noqa: E402
from concourse.tile import TileContext  # noqa: E402

F32 = mybir.dt.float32
F16 = mybir.dt.float16
F8 = mybir.dt.float8e4
ALU = mybir.AluOpType
AF = mybir.ActivationFunctionType
PM = mybir.MatmulPerfMode

B, D, K, O = 8192, 64, 64, 64
NCORES = 8
BSH = B // NCORES          # 1024 batch rows per core
NCHUNK = 2                 # 512-column matmul chunks
CHUNK = BSH // NCHUNK      # 512
NS = K - 1                 # 63 segments
NGRP = 32                  # groups of 2 segments (last half padded)
X_MIN, X_MAX = -2.0, 2.0
H = (X_MAX - X_MIN) / (K - 1)

CTB = NGRP * 3 * O         # 6144 table cols: per group [b | c | d] x O
TB_SPLIT = 8 * 3 * O       # first-chunk table DMA (groups 0..7)

# sb const tensor [128, 34] fp32: cols 0..31 group biases (-s per partition),
# col 32 = -(K-1) edge-hi bias, col 33 = 0.0 edge-lo bias
SB_EHI = 32
SB_ELO = 33
CSB = 34

WORK_BUFS = 4
WARM_N = 9                 # PE p-state warm matmuls bridging the DMA wait
EDGE_AT = 8                # group index after which edge fields are built
U3_DVE = {2, 4, 6, 10, 12, 14, 18, 20, 22, 26, 28, 30}  # u3 on DVE
U2_POOL = set()            # fp16 groups whose u2 runs on Pool to unload DVE
Y_DVE = set()              # groups whose y runs on DVE (ts, 2x_2p) not ACT
Y_POOL = set()             # groups whose y runs on Pool
# Odd groups chain y from the previous group's tile: y_j = y_{j-1} - 2
# (same partition layout), a 327ns DVE ts-op instead of a 1038ns ACT op.
# The freed ACT slots take even groups' u2 as Square(u).
# chain map: j -> (source group, delta); even groups seed from ACT, odd
# groups chain y_j = y_{j-1} - 2 as a 327ns DVE ts-op (deeper chains
# regress the pipeline cadence)
Y_CHAIN = {j: (j - 1, -2.0) for j in range(1, 32, 2)}
U2_ACT = frozenset(range(0, 32, 2))
USE_POW = False            # pow not supported by walrus codegen
# Groups evaluated via fp8-e4m3 DoubleRow: fields q=u(u-1), r=q*u (zero on
# both plateaus, so fp8 tables only touch the locally-active segment) with
# tables (c+d, d); the u-field stays fp16 with table dC. Interleaved with
# fp16 groups so Pool's two fp8 writes per DR group pipeline against ACT's
# y cadence.
DR_GROUPS = frozenset(range(1, 32, 2))
NDR = len(DR_GROUPS)
# group emission order: group 0 must stay first (PSUM start + halved DMA
# wait); ending on an fp16 group whose u3 is on DVE keeps Pool off the
# final dependency chain
GROUP_ORDER = list(range(32))
# engines for the four 256-col output pieces (ACT / DVE / Pool)
OUT_ENGINES = ("act", "dve", "dve", "act")
OUT_BOUNDS = (0, 320, 512, 704, 1024)
OUT_DMA_Q = ("pool", "sp", "sp", "act")

TRACE = False
LAST_EXEC_NS = None


def _pchip_slopes_uniform(y, h):
    """numpy float32 port of reference._pchip_slopes_uniform. y: [..., K]."""
    y = y.astype(np.float32)
    delta = ((y[..., 1:] - y[..., :-1]) / np.float32(h)).astype(np.float32)
    dp, dn = delta[..., :-1], delta[..., 1:]
    same_sign = dp * dn > 0
    d_mid = np.where(
        same_sign, (2.0 * dp * dn / (dp + dn + np.float32(1e-12))), np.float32(0.0)
    ).astype(np.float32)

    def _fix_endpoint(d_end, delta0, delta1):
        d_end = np.where(d_end * delta0 <= 0, np.float32(0.0), d_end)
        d_end = np.where(
            (delta0 * delta1 < 0) & (np.abs(d_end) > 3.0 * np.abs(delta0)),
            (3.0 * delta0).astype(np.float32),
            d_end,
        )
        return d_end.astype(np.float32)

    d0 = _fix_endpoint(
        ((3.0 * delta[..., 0] - delta[..., 1]) / 2.0).astype(np.float32),
        delta[..., 0],
        delta[..., 1],
    )
    dN = _fix_endpoint(
        ((3.0 * delta[..., -1] - delta[..., -2]) / 2.0).astype(np.float32),
        delta[..., -1],
        delta[..., -2],
    )
    return np.concatenate([d0[..., None], d_mid, dN[..., None]], axis=-1)


def _build_kernel():
    nc = bass.Bass()

    t2 = nc.declare_dram_parameter("t2", [128, BSH], F32, isOutput=False)
    tb = nc.declare_dram_parameter("tb", [128, CTB], F16, isOutput=False)
    tb8 = nc.declare_dram_parameter("tb8", [128, 2, NDR * O], F8, isOutput=False)
    etab = nc.declare_dram_parameter("etab", [128, O], F16, isOutput=False)
    sb = nc.declare_dram_parameter("sb", [128, CSB], F32, isOutput=False)
    k0 = nc.declare_dram_parameter("k0", [O, 1], F32, isOutput=False)
    outt = nc.declare_dram_parameter("outt", [O, BSH], F32, isOutput=True)

    with TileContext(nc) as tc:
        with (
            tc.tile_pool(name="consts", bufs=1) as consts,
            tc.tile_pool(name="work", bufs=WORK_BUFS) as work,
            tc.tile_pool(name="accp", bufs=1, space="PSUM") as accp,
        ):
            t2_sb = consts.tile([128, BSH], F32)
            tb_sb = consts.tile([128, CTB], F16)
            tb8_sb = consts.tile([128, 2, NDR * O], F8)
            etab_sb = consts.tile([128, O], F16)
            sb_sb = consts.tile([128, CSB], F32)
            k0_sb = consts.tile([O, 1], F32)
            # sb + first table chunk serially on the SP queue; t2 halves on
            # the DVE/ACT queues in parallel so group 0 starts ~1us earlier
            nc.sync.dma_start(sb_sb[:], sb[:])
            nc.scalar.dma_start(t2_sb[:, 0:CHUNK], t2[:, 0:CHUNK])
            nc.gpsimd.dma_start(t2_sb[:, CHUNK:], t2[:, CHUNK:])
            nc.sync.dma_start(tb_sb[:, :TB_SPLIT], tb[:, :TB_SPLIT])
            nc.sync.dma_start(tb8_sb[:], tb8[:])
            # rest of the fp16 tables in two pieces so groups 8..19 aren't
            # stuck behind one monolithic transfer on the serial SP queue
            TB_MID = 20 * 3 * O
            nc.sync.dma_start(tb_sb[:, TB_SPLIT:TB_MID], tb[:, TB_SPLIT:TB_MID])
            nc.sync.dma_start(tb_sb[:, TB_MID:], tb[:, TB_MID:])
            nc.sync.dma_start(etab_sb[:], etab[:])
            nc.sync.dma_start(k0_sb[:], k0[:])

            dr_list = sorted(DR_GROUPS)

            def grp_tab(j, f):
                lo = j * 3 * O + f * O
                return tb_sb[:, lo : lo + O]

            def dr_tab(j):
                gi = dr_list.index(j)
                return tb8_sb[:, :, gi * O : (gi + 1) * O]

            # PSUM accumulator [O, 1024] (2 banks). Warm matmuls keep the PE
            # p-state ramp going from t=0 on a memset tile; results are
            # discarded by the start=True restarts below.
            # one PSUM tile per 512-col chunk so chunk 0's output path does
            # not serialize behind chunk 1's accumulation (tile-granularity
            # dependency tracking)
            acc0 = accp.tile([O, CHUNK], F32)
            acc1 = accp.tile([O, CHUNK], F32)
            accs = [acc0, acc1]
            warm = consts.tile([128, 512], F16, tag="warm")
            # preload the activation-function table before t2 arrives so the
            # first y doesn't pay the 1283ns table load; feed it from a tiny
            # memset tile so it doesn't wait for the full warm-tile memset
            dummy_in = consts.tile([1, 1], F16, tag="dummy_in")
            nc.vector.memset(dummy_in[:], 0.0)
            nc.vector.memset(warm[:], 0.0)
            dummy = consts.tile([1, 1], F16, tag="dummy")
            nc.scalar.activation(dummy[:], dummy_in[:], AF.Identity)
            for _ in range(WARM_N):
                nc.tensor.matmul(
                    acc0[0:64, 0:512],
                    warm[:, 0:64],
                    warm[:, 0:512],
                    start=True,
                    stop=True,
                )

            # edge (extrapolation) fields, built on Pool in its idle window
            # right after the t2 DMA: rows 0:64 = relu(-t) -> -hS[d,0],
            # rows 64:128 = relu(t-63) -> hS[d,63]
            edges = consts.tile([128, BSH], F16, tag="edges")
            nc.gpsimd.tensor_scalar(
                edges[0:64, :], t2_sb[0:64, :], -1.0, 0.0, ALU.mult, ALU.max
            )
            nc.gpsimd.tensor_scalar(
                edges[64:128, :], t2_sb[64:128, :], float(-(K - 1)), 0.0,
                ALU.add, ALU.max,
            )
            obs = []
            for q in range(4):
                ob_q = consts.tile(
                    [O, OUT_BOUNDS[q + 1] - OUT_BOUNDS[q]], F32,
                    tag=f"ob{q}", name=f"ob{q}",
                )
                obs.append(ob_q)

            ytiles = {}
            for gidx, j in enumerate(GROUP_ORDER):
                is_dr = j in DR_GROUPS
                y = work.tile([128, BSH], F16, tag="y")
                u = work.tile([128, BSH], F16, tag="u")
                if is_dr:
                    qa = work.tile([128, BSH], F16, tag="qa")
                    qr = work.tile([128, 2, BSH], F8, tag="qr")
                else:
                    u2 = work.tile([128, BSH], F16, tag="u2")
                    u3 = work.tile([128, BSH], F16, tag="u3")
                # group 0 is built in column halves so its first matmuls only
                # wait on the first half of the t2 DMA
                halves = (
                    [slice(0, CHUNK), slice(CHUNK, BSH)]
                    if gidx == 0
                    else [slice(0, BSH)]
                )
                for h in halves:
                    if j in Y_CHAIN and Y_CHAIN[j][0] in ytiles:
                        src_j, delta = Y_CHAIN[j]
                        nc.vector.tensor_scalar(
                            y[:, h], ytiles[src_j][:, h], delta, None, ALU.add
                        )
                    elif j in Y_DVE:
                        nc.vector.tensor_scalar(
                            y[:, h], t2_sb[:, h], sb_sb[:, j : j + 1], None,
                            ALU.add,
                        )
                    elif j in Y_POOL:
                        nc.gpsimd.tensor_scalar(
                            y[:, h], t2_sb[:, h], sb_sb[:, j : j + 1], None,
                            ALU.add,
                        )
                    else:
                        nc.scalar.activation(
                            y[:, h], t2_sb[:, h], AF.Identity,
                            bias=sb_sb[:, j : j + 1], scale=1.0,
                        )
                    nc.vector.tensor_scalar(
                        u[:, h], y[:, h], 0.0, 1.0, ALU.max, ALU.min
                    )
                    if is_dr:
                        nc.vector.tensor_scalar(qa[:, h], u[:, h], -1.0, None, ALU.add)
                        nc.gpsimd.tensor_tensor(
                            qr[:, 0, h], u[:, h], qa[:, h], ALU.mult
                        )
                        nc.gpsimd.tensor_tensor(
                            qr[:, 1, h], qr[:, 0, h], u[:, h], ALU.mult
                        )
                    elif j in U2_ACT and not is_dr:
                        nc.scalar.activation(u2[:, h], u[:, h], AF.Square)
                        if j in U3_DVE:
                            nc.vector.tensor_tensor(
                                u3[:, h], u2[:, h], u[:, h], ALU.mult
                            )
                        else:
                            nc.gpsimd.tensor_tensor(
                                u3[:, h], u2[:, h], u[:, h], ALU.mult
                            )
                    elif USE_POW:
                        nc.vector.tensor_scalar(u2[:, h], u[:, h], 2.0, None, ALU.pow)
                        if j in U3_DVE:
                            nc.vector.tensor_scalar(
                                u3[:, h], u[:, h], 3.0, None, ALU.pow
                            )
                        else:
                            nc.gpsimd.tensor_tensor(
                                u3[:, h], u2[:, h], u[:, h], ALU.mult
                            )
                    elif j in U2_POOL:
                        nc.gpsimd.tensor_tensor(u2[:, h], u[:, h], u[:, h], ALU.mult)
                        nc.gpsimd.tensor_tensor(u3[:, h], u2[:, h], u[:, h], ALU.mult)
                    else:
                        nc.vector.tensor_tensor(u2[:, h], u[:, h], u[:, h], ALU.mult)
                        if j in U3_DVE:
                            nc.vector.tensor_tensor(
                                u3[:, h], u2[:, h], u[:, h], ALU.mult
                            )
                        else:
                            nc.gpsimd.tensor_tensor(
                                u3[:, h], u2[:, h], u[:, h], ALU.mult
                            )

                ytiles[j] = y

                last = gidx == NGRP - 1
                for c in range(NCHUNK):
                    sl = slice(c * CHUNK, (c + 1) * CHUNK)
                    nc.tensor.matmul(
                        accs[c][:], grp_tab(j, 0), u[:, sl],
                        start=(gidx == 0), stop=False,
                    )
                    if gidx == 1:
                        # edge matmuls accumulate early so the finale only
                        # waits on the last group's own fields
                        nc.tensor.matmul(
                            accs[c][:], etab_sb[:], edges[:, sl],
                            start=False, stop=False,
                        )
                    if is_dr:
                        nc.tensor.matmul(
                            accs[c][:], dr_tab(j), qr[:, :, sl],
                            start=False, stop=last, perf_mode=PM.DoubleRow,
                        )
                    else:
                        nc.tensor.matmul(
                            accs[c][:], grp_tab(j, 1), u2[:, sl],
                            start=False, stop=False,
                        )
                        nc.tensor.matmul(
                            accs[c][:], grp_tab(j, 2), u3[:, sl],
                            start=False, stop=last,
                        )

            # bias/const add + DMA out in 256-col pieces, after ALL matmuls
            # (acc is one tile: an early read would add a write-after-read
            # stall on the remaining accumulation). Separate ob tiles so the
            # four pieces don't serialize; DMAs spread across queues.
            dma_map = {"sp": nc.sync, "pool": nc.gpsimd, "act": nc.scalar}
            dma_eng = [dma_map[e] for e in OUT_DMA_Q]
            # piece boundaries: last piece smallest so the final DMA (on the
            # critical path) has the shortest transfer
            bounds = OUT_BOUNDS
            for q in range(4):
                qsl = slice(bounds[q], bounds[q + 1])
                asl = slice(bounds[q] % CHUNK, ((bounds[q + 1] - 1) % CHUNK) + 1)
                eng = OUT_ENGINES[q]
                if eng == "act":
                    nc.scalar.activation(
                        obs[q][:], accs[q // 2][:, asl], AF.Identity,
                        bias=k0_sb[:, 0:1], scale=1.0,
                    )
                elif eng == "dve":
                    nc.vector.tensor_scalar(
                        obs[q][:], accs[q // 2][:, asl], k0_sb[:, 0:1], None, ALU.add
                    )
                else:
                    nc.gpsimd.tensor_scalar(
                        obs[q][:], accs[q // 2][:, asl], k0_sb[:, 0:1], None, ALU.add
                    )
                dma_eng[q].dma_start(outt[:, qsl], obs[q][:])

    _split_multiwaits(nc)
    return nc


def _split_multiwaits(nc):
    """walrus (neuronx-cc) allows one sync wait per instruction; move extra
    waits onto standalone NoOps inserted just before the offender."""
    cnt = 0
    for f in nc.m.functions:
        for blk in f.blocks:
            out = []
            changed = False
            for ins in blk.instructions:
                si = ins.sync_info
                if si is not None and len(si.on_wait) > 1:
                    waits = list(si.on_wait)
                    for w in waits[:-1]:
                        nop = mybir.InstNoOp(name=f"I-ws-{cnt}", ins=[], outs=[])
                        cnt += 1
                        nop.engine = ins.engine
                        nop.sync_info = type(si)(on_wait=[w], on_update=[])
                        out.append(nop)
                    ins.sync_info = type(si)(
                        on_wait=[waits[-1]], on_update=list(si.on_update)
                    )
                    changed = True
                out.append(ins)
            if changed:
                blk.instructions = out


def _host_tables(coeffs, bias):
    coeffs = np.ascontiguousarray(np.asarray(coeffs, dtype=np.float32))
    bias = np.asarray(bias, dtype=np.float32)
    slopes = _pchip_slopes_uniform(coeffs, H)          # [O, D, K]
    hs = (slopes * np.float32(H)).astype(np.float32)   # h * S

    C = coeffs
    dC = C[..., 1:] - C[..., :-1]                      # [O, D, NS]
    c = (3.0 * dC - 2.0 * hs[..., :-1] - hs[..., 1:]).astype(np.float32)
    d = (-2.0 * dC + hs[..., :-1] + hs[..., 1:]).astype(np.float32)
    c16 = c.astype(np.float16)
    d16 = d.astype(np.float16)
    # compensate b so the u=1 plateau sum b+c+d telescopes to dC as exactly
    # as fp16 allows
    b16 = (dC - c16.astype(np.float32) - d16.astype(np.float32)).astype(np.float16)

    from ml_dtypes import float8_e4m3fn as E4M3

    tb = np.zeros((128, CTB), dtype=np.float16)
    tb8v = np.zeros((128, 2, NDR * O), dtype=np.float32)
    dr_list = sorted(DR_GROUPS)
    tabs = (b16, c16, d16)
    for j in range(NGRP):
        is_dr = j in DR_GROUPS
        for half in range(2):
            s = 2 * j + half
            if s >= NS:
                continue
            rows = slice(half * 64, (half + 1) * 64)
            if is_dr:
                # u-field table = dC (plateau-exact); q,r tables in fp8
                gi = dr_list.index(j)
                lo = j * 3 * O
                tb[rows, lo : lo + O] = dC[:, :, s].T.astype(np.float16)
                tb8v[rows, 0, gi * O : (gi + 1) * O] = (c + d)[:, :, s].T
                tb8v[rows, 1, gi * O : (gi + 1) * O] = d[:, :, s].T
            else:
                for f in range(3):
                    lo = j * 3 * O + f * O
                    # rows = dims, cols = o
                    tb[rows, lo : lo + O] = tabs[f][:, :, s].T
    tb8 = tb8v.astype(E4M3)

    etab = np.zeros((128, O), dtype=np.float16)
    etab[0:64, :] = -hs[:, :, 0].T
    etab[64:128, :] = hs[:, :, K - 1].T

    sb = np.zeros((128, CSB), dtype=np.float32)
    for j in range(NGRP):
        sb[0:64, j] = -(2 * j)
        sb[64:128, j] = -(2 * j + 1)
    sb[:, SB_EHI] = -(K - 1)
    sb[:, SB_ELO] = 0.0

    k0 = (C[..., 0].sum(axis=1) + bias).astype(np.float32).reshape(O, 1)
    return tb, tb8, etab, sb, k0


def kernel(x, coeffs, bias):
    global LAST_EXEC_NS
    x = np.asarray(x, dtype=np.float32)
    tb, tb8, etab, sb, k0 = _host_tables(coeffs, bias)

    in_maps = []
    for r in range(NCORES):
        xc = x[r * BSH : (r + 1) * BSH, :]             # [1024, 64]
        t = ((xc.T - np.float32(X_MIN)) * np.float32(1.0 / H)).astype(np.float32)
        t2 = np.ascontiguousarray(np.concatenate([t, t], axis=0))  # [128, 1024]
        in_maps.append(
            {"t2": t2, "tb": tb, "tb8": tb8, "etab": etab, "sb": sb, "k0": k0}
        )

    nc = _build_kernel()
    res = run_bass_kernel_spmd(nc, in_maps, list(range(NCORES)), trace=TRACE)
    LAST_EXEC_NS = getattr(res, "exec_time_ns", None)

    out = np.empty((B, O), dtype=np.float32)
    for r in range(NCORES):
        out_t = res.results[r]["outt"]                 # [O, 1024]
        out[r * BSH : (r + 1) * BSH, :] = np.asarray(out_t).T
    return out


if __name__ == "__main__":
    rng = np.random.default_rng(0)
    x = rng.standard_normal((B, D)).astype(np.float32)
    coeffs = (0.01 * rng.standard_normal((O, D, K))).astype(np.float32)
    bias = np.zeros((O,), dtype=np.float32)
    out = kernel(x, coeffs, bias)
    print("out", out.shape, out.dtype, float(np.abs(out).mean()))
